# revision 22
# baseline (speedup 1.0000x reference)
"""Trainium2 Bass kernel for a single transformer block (nn_Block_3212635537783).

Reference computation (B=4, T=2048, C=768, H=12, D=64):
    q/k/v per-head projections of x; scores[t,s] = k[t]@q[s]/sqrt(C) with
    causal mask (s <= t), softmax over s; a[t] = sum_s w[t,s] v[s];
    x = LN1(x + a); x = LN2(x + gelu(x@W1 + b1)@W2 + b2)

Sharding: 8 cores = 4 batches x 2 token-interleaved halves. Core (b, g)
owns rows {g, g+2, ...} of batch b. The stride-2 interleave keeps the
causal workload balanced AND the SPMD program identical across cores
(only input data differs; the +-1 row causal boundary lives in a tiny
per-core mask tile).

On-chip layout is fully "transposed": activations are [C, tokens]
(feature dim on partitions) so attention, layernorm and the MLP never
need an on-chip transpose. Matmul inputs are bf16 (fp32 accumulation).

v2 scheduling notes:
 - AV matmuls are issued one s-chunk behind the score matmuls so the PE
   never waits in-line on the ACT exp of the same chunk.
 - softmax denominators: 1/den = exp(-ln(den)) on ACT (ln shares the
   exp activation table: zero table swaps) -> rank-1 PE broadcast into
   the unused upper rows of the av PSUM bank -> DVE stage + multiply;
   the chain is emitted *after* the next token-block's first scores so
   its latency hides under real PE work.
 - layernorm: bf16 residual stream; stats via ones-matmuls (Square on
   ACT); rsqrt = exp(-0.5*ln(var+eps)) on ACT; (x*A - B) form where
   A = 1 (x) rs, B = 1 (x) mu*rs ride two rank-1 PE broadcasts read
   directly from PSUM by the DVE (no ACT staging copies).
 - all HBM loads are issued from the (otherwise idle) GpSimd sequencer
   (25ns dispatch vs 565ns on Sync), most-urgent tiles first.
"""

import sys
import types

import numpy as np
import ml_dtypes

B, T, C, H, D = 4, 2048, 768, 12, 64
F = 4 * C            # 3072
P = 128              # partitions
OT = T // 2          # owned tokens per core (1024)
NB_C = C // P        # 6 c-chunks
NB_F = F // P        # 24 hidden chunks
NPAIR = H // 2       # 6 head-pair chunks
EPS = 1e-5
SCALE = float(1.0 / np.sqrt(np.float32(C)))
N_CORES = 8
HG = 4               # heads per attention group
N_HG = H // HG       # 3 groups

BF16 = ml_dtypes.bfloat16

_compiled = {}


# --------------------------------------------------------------------------
# environment patches (must live in kernel.py: the grader imports only this
# file). Idempotent.
# --------------------------------------------------------------------------

def _patch_tile_drain():
    """This walrus build rejects >1 sync-wait command on the final Tile
    drain CTRL instruction. Spread the drain's waits across chained
    sync-engine nops (same engine => program order preserved; the
    all-engine barrier after them still gates the semaphore clears)."""
    import concourse.tile as tile_mod
    import concourse.mybir as mybir

    if getattr(tile_mod.TileContext, "_drain_patched", False):
        return

    def patched(self, tick_clock, wait_clock):
        from concourse.vector_clock import ScopedClock

        drain_inst = self.nc.sync.drain()
        wait_clock.add_sem_waits(
            drain_inst.ins, ScopedClock({None: tick_clock.global_clock})
        )
        si = drain_inst.ins.sync_info
        waits = list(si.on_wait) if si else []
        MAXW = 1
        if len(waits) > MAXW:
            si.on_wait = waits[:MAXW]
            rest = waits[MAXW:]
            while rest:
                nop = self.nc.sync.nop(nofuse=True)
                chunk, rest = rest[:MAXW], rest[MAXW:]
                nsi = nop.ins.sync_info
                if nsi is None:
                    nop.ins.sync_info = mybir.SyncInfo(on_wait=chunk, on_update=[])
                else:
                    nsi.on_wait = list(nsi.on_wait) + chunk
        self.nc.all_engine_barrier()
        assert self.sems is not None
        popped = self.nc._tile_sem_poison_stack.pop()
        assert popped is self._sem_poison
        self.nc.clear_and_free_semaphores(list(self.sems.allocated().values()))
        self.nc.all_engine_barrier()

    tile_mod.TileContext._drain_and_barrier = patched
    tile_mod.TileContext._drain_patched = True


def _patch_profile_hook():
    """Optional: register the axon NTFF profiling hook so trace=True works
    (used for timing; harmless no-op if unavailable)."""
    if "antenv.axon_hooks" in sys.modules:
        return
    try:
        sys.path.insert(0, "/root/.axon_site")
        from trn_agent_boot.trn_boot import _ntff_profile_via_ctypes

        hook = _ntff_profile_via_ctypes("/opt/axon/libaxon_pjrt.so")
        mod = types.ModuleType("antenv.axon_hooks")
        mod.get_axon_ntff_profile_hook = lambda: hook
        mod.set_axon_ntff_profile_hook = lambda h: None
        sys.modules["antenv.axon_hooks"] = mod
        import concourse.bass_utils as bu

        bu.upload_artifacts = lambda tmpdir: "local://" + tmpdir
    except Exception:
        pass


# --------------------------------------------------------------------------
# program construction (shared by all 8 cores; SPMD over input data)
# --------------------------------------------------------------------------

def _build_nc(trivial_gb):
    import contextlib

    import concourse.bass as bass
    import concourse.mybir as mybir
    from concourse.tile import TileContext

    f32 = mybir.dt.float32
    f32r = mybir.dt.float32r
    bf16 = mybir.dt.bfloat16
    ALU = mybir.AluOpType
    AF = mybir.ActivationFunctionType

    nc = bass.Bass()

    # ---- DRAM I/O ----
    xT = nc.declare_dram_parameter("xT", [C, T], bf16, isOutput=False)
    xTo16 = nc.declare_dram_parameter("xTo16", [C, OT], bf16, isOutput=False)
    wq = nc.declare_dram_parameter("wq", [C, C], bf16, isOutput=False)
    wk = nc.declare_dram_parameter("wk", [C, C], bf16, isOutput=False)
    wv = nc.declare_dram_parameter("wv", [C, C], bf16, isOutput=False)
    w1 = nc.declare_dram_parameter("w1", [C, F], bf16, isOutput=False)
    w2 = nc.declare_dram_parameter("w2", [F, C], bf16, isOutput=False)
    b1r = nc.declare_dram_parameter("b1r", [P, NB_F], f32, isOutput=False)
    b2r = nc.declare_dram_parameter("b2r", [P, NB_C], f32, isOutput=False)
    g1r = nc.declare_dram_parameter("g1r", [P, NB_C], f32, isOutput=False)
    be1r = nc.declare_dram_parameter("be1r", [P, NB_C], f32, isOutput=False)
    g2r = nc.declare_dram_parameter("g2r", [P, NB_C], f32, isOutput=False)
    be2r = nc.declare_dram_parameter("be2r", [P, NB_C], f32, isOutput=False)
    cmask = nc.declare_dram_parameter("cmask", [P, 64], bf16, isOutput=False)
    outT = nc.declare_dram_parameter("outT", [C, OT], f32, isOutput=True)

    xT_t = xT[:].rearrange("(n p) t -> n p t", p=P)
    xTo16_t = xTo16[:].rearrange("(n p) t -> n p t", p=P)
    wq_t = wq[:].rearrange("(n p) c -> n p c", p=P)
    wk_t = wk[:].rearrange("(n p) c -> n p c", p=P)
    wv_t = wv[:].rearrange("(n p) c -> n p c", p=P)
    w1_t = w1[:].rearrange("(n p) f -> n p f", p=P)
    w2_t = w2[:].rearrange("(n p) c -> n p c", p=P)
    outT_t = outT[:].rearrange("(n p) t -> n p t", p=P)

    with TileContext(nc) as tc, contextlib.ExitStack() as ctx:
        const = ctx.enter_context(tc.tile_pool(name="const", bufs=1))
        p_a = ctx.enter_context(tc.tile_pool(name="attn_a", bufs=1))
        p_xo = ctx.enter_context(tc.tile_pool(name="xo", bufs=1))
        p_mlpw = ctx.enter_context(tc.tile_pool(name="mlpw", bufs=1))
        import contextlib as _ctl
        xt_stack = _ctl.ExitStack()
        p_xt = xt_stack.enter_context(tc.tile_pool(name="xt", bufs=1))

        # ---- constants (memsets now; const DMAs issued later on gpsimd
        # so the critical first loads go out first) ----
        ones_k = const.tile([P, 1], bf16, tag="ones_k", name="ones_k")
        nc.vector.memset(ones_k, 1.0)
        ones_row = const.tile([1, P], bf16, tag="ones_row", name="ones_row")
        nc.vector.memset(ones_row, 1.0)
        eps_t = const.tile([1, 1], f32, tag="eps", name="eps_t")
        nc.vector.memset(eps_t, EPS)
        msk = const.tile([P, 64], bf16, tag="msk", name="msk")
        msk2 = bass.AP(
            tensor=msk.tensor, offset=msk.offset,
            ap=[list(msk.ap[0]), [0, 2], list(msk.ap[1])],
        )
        sb_b1 = const.tile([P, NB_F], f32, tag="b1", name="sb_b1")
        sb_b2 = const.tile([P, NB_C], f32, tag="b2", name="sb_b2")
        sb_g1 = const.tile([P, NB_C], f32, tag="g1", name="sb_g1")
        sb_be1 = const.tile([P, NB_C], f32, tag="be1", name="sb_be1")
        sb_g2 = const.tile([P, NB_C], f32, tag="g2", name="sb_g2")
        sb_be2 = const.tile([P, NB_C], f32, tag="be2", name="sb_be2")

        # ---- persistent activation tiles ----
        sb_xt = [
            p_xt.tile([P, T], bf16, tag=f"xt{k}", name=f"xt{k}")
            for k in range(NB_C)
        ]
        sb_xto16 = [
            p_xo.tile([P, OT], bf16, tag=f"xto16_{k}", name=f"xto16_{k}")
            for k in range(NB_C)
        ]

        # attention output a^T, bf16 [128, OT] per pair-chunk
        sb_a = [
            p_a.tile([P, OT], bf16, tag=f"a{pc}", name=f"a{pc}")
            for pc in range(NPAIR)
        ]

        # MLP weight tiles (DMA'd later, after attention weights)
        sb_w1 = [
            p_mlpw.tile([P, F], bf16, tag=f"w1_{k}", name=f"w1_{k}")
            for k in range(NB_C)
        ]
        sb_w2 = [
            p_mlpw.tile([P, C], bf16, tag=f"w2_{m}", name=f"w2_{m}")
            for m in range(NB_F)
        ]

        # ============================================================
        # Phase A: attention, in head groups of HG. All attention pools
        # are global (slot-ring tags) so projection work for group hg+1
        # can interleave into attention of group hg.
        # ============================================================
        attn_stack = _ctl.ExitStack()
        p_ps = attn_stack.enter_context(
            tc.tile_pool(name="attnps", bufs=1, space="PSUM")
        )
        p_dn = attn_stack.enter_context(tc.tile_pool(name="dn", bufs=1))
        p_w = attn_stack.enter_context(tc.tile_pool(name="wqk", bufs=1))
        p_qk = attn_stack.enter_context(tc.tile_pool(name="qk", bufs=1))
        p_v = attn_stack.enter_context(tc.tile_pool(name="vv", bufs=1))
        p_e = attn_stack.enter_context(tc.tile_pool(name="ee", bufs=1))

        def group_pcs(hg):
            return [hg * (HG // 2) + i for i in range(HG // 2)]

        wq_sb, wk_sb, q_t, k_t = {}, {}, {}, {}

        def issue_qk_dmas(hg, eng):
            """Allocate weight/output tiles for group hg's q/k projections
            and queue the weight loads on `eng`'s sequencer."""
            for pc in group_pcs(hg):
                wq_sb[pc] = []
                for k in range(NB_C):
                    wt = p_w.tile(
                        [P, P], bf16, tag=f"wq{pc % 4}_{k}",
                        name=f"wq{pc}_{k}"
                    )
                    eng.dma_start(
                        out=wt, in_=wq_t[k][:, pc * P : (pc + 1) * P]
                    )
                    wq_sb[pc].append(wt)
            for pc in group_pcs(hg):
                wk_sb[pc] = []
                for k in range(NB_C):
                    wt = p_w.tile(
                        [P, P], bf16, tag=f"wk{pc % 4}_{k}",
                        name=f"wk{pc}_{k}"
                    )
                    eng.dma_start(
                        out=wt, in_=wk_t[k][:, pc * P : (pc + 1) * P]
                    )
                    wk_sb[pc].append(wt)

        def qk_units(hg):
            """Generator: one closure per projection psum-tile for group
            hg's q/k projections (8 q units + 4 k units)."""
            for pc in group_pcs(hg):
                q_t[pc] = p_qk.tile(
                    [P, T], bf16, tag=f"q{pc % 4}", name=f"q{pc}"
                )
                k_t[pc] = p_qk.tile(
                    [P, OT], bf16, tag=f"k{pc % 4}", name=f"k{pc}"
                )
            for pc in group_pcs(hg):
                for t4 in range(T // 512):
                    def qu(pc=pc, t4=t4):
                        ps = p_ps.tile(
                            [P, 512], f32, tag="ps", bufs=2, name="ps_prj"
                        )
                        for k in range(NB_C):
                            nc.tensor.matmul(
                                ps,
                                wq_sb[pc][k],
                                sb_xt[k][:, t4 * 512 : (t4 + 1) * 512],
                                start=(k == 0),
                                stop=(k == NB_C - 1),
                            )
                        nc.vector.tensor_copy(
                            q_t[pc][:, t4 * 512 : (t4 + 1) * 512], ps
                        )
                    yield qu
                for t2 in range(OT // 512):
                    def ku(pc=pc, t2=t2):
                        ps = p_ps.tile(
                            [P, 512], f32, tag="ps", bufs=2, name="ps_prk"
                        )
                        for k in range(NB_C):
                            nc.tensor.matmul(
                                ps,
                                wk_sb[pc][k],
                                sb_xto16[k][:, t2 * 512 : (t2 + 1) * 512],
                                start=(k == 0),
                                stop=(k == NB_C - 1),
                            )
                        nc.vector.tensor_copy(
                            k_t[pc][:, t2 * 512 : (t2 + 1) * 512], ps
                        )
                    yield ku

        # ---- DMA issue: critical path (group-0 q weights + first xT
        # quarter) on the Sync sequencer; everything else on GpSimd in
        # urgency order. Both sequencers issue in parallel. ----
        issue_qk_dmas(0, nc.sync)
        for k in range(NB_C):
            nc.sync.dma_start(out=sb_xt[k][:, 0:512], in_=xT_t[k][:, 0:512])
        for k in range(NB_C):
            nc.gpsimd.dma_start(out=sb_xto16[k], in_=xTo16_t[k])
        for quarter in range(1, 4):
            cs = slice(quarter * 512, (quarter + 1) * 512)
            for k in range(NB_C):
                nc.gpsimd.dma_start(out=sb_xt[k][:, cs], in_=xT_t[k][:, cs])
        nc.gpsimd.dma_start(out=msk, in_=cmask[:])
        nc.gpsimd.dma_start(out=sb_b1, in_=b1r[:])
        nc.gpsimd.dma_start(out=sb_b2, in_=b2r[:])
        nc.gpsimd.dma_start(out=sb_g1, in_=g1r[:])
        nc.gpsimd.dma_start(out=sb_be1, in_=be1r[:])
        nc.gpsimd.dma_start(out=sb_g2, in_=g2r[:])
        nc.gpsimd.dma_start(out=sb_be2, in_=be2r[:])

        def make_norm(tb, pcs, av):
            """Softmax normalization for one token-block. emit_copy frees
            the av PSUM banks with 4 cheap DVE copies (all the new block's
            AV accumulation waits on); the reciprocal chain in emit_rest
            runs entirely off the critical path against the copies."""
            av_sb = {}
            heads = [2 * pc + j for pc in pcs for j in range(2)]

            def emit_copy():
                for h in heads:
                    t = p_dn.tile([65, 512], bf16, tag="avsb", bufs=8,
                                  name="av_sb")
                    nc.vector.tensor_copy(t, av[h][0:65, 0:512])
                    av_sb[h] = t

            recs = {}

            def emit_recs():
                # 1/den = exp(-ln(den)) on ACT (ln shares exp's table)
                for h in heads:
                    lnd = p_dn.tile([1, 512], f32, tag="lnd", bufs=4,
                                    name="lnd")
                    nc.scalar.activation(
                        out=lnd, in_=av_sb[h][64:65, :], func=AF.Ln
                    )
                    rec = p_dn.tile([1, 512], bf16, tag="rec", bufs=4,
                                    name="rec")
                    nc.scalar.activation(
                        out=rec, in_=lnd, func=AF.Exp, scale=-1.0
                    )
                    recs[h] = rec

            def emit_bm():
                # rank-1 broadcast of 1/den + normalize multiply. Emitted
                # well after emit_recs so the borrowed score-ring PSUM
                # slot is held only briefly.
                dps = p_ps.tile([P, 2, 512], f32, tag="ps", bufs=2,
                                name="den_ps")
                for j, pc in enumerate(pcs):
                    for par in range(2):
                        nc.tensor.matmul(
                            dps[par * 64 : par * 64 + 64, j, :],
                            ones_row[:, 0:64],
                            recs[2 * pc + par],
                            start=True, stop=True,
                        )
                for j, pc in enumerate(pcs):
                    for par in range(2):
                        h = 2 * pc + par
                        nc.vector.tensor_tensor(
                            sb_a[pc][par * 64 : par * 64 + 64,
                                     tb * 512 : (tb + 1) * 512],
                            av_sb[h][0:64, :],
                            dps[par * 64 : par * 64 + 64, j, :],
                            ALU.mult,
                        )

            return emit_copy, emit_recs, emit_bm

        pending_norm = []   # (due_tick, closure), ascending
        norm_clock = [0]

        def queue_norm(emit_recs, emit_bm):
            pending_norm.append((norm_clock[0] + 2, emit_recs))
            pending_norm.append((norm_clock[0] + 6, emit_bm))

        def norm_tick():
            norm_clock[0] += 1
            while pending_norm and pending_norm[0][0] <= norm_clock[0]:
                pending_norm.pop(0)[1]()

        for hg in range(N_HG):
            pcs = group_pcs(hg)
            heads = [2 * pc + j for pc in pcs for j in range(2)]

            _sc_p = nc.enter_named_scope(f"proj{hg}", False)
            if hg == 0:
                for u in qk_units(0):
                    u()

            # ---- v projection block for this group ----
            d0 = heads[0] * D
            wvl = []
            for k in range(NB_C):
                wt = p_w.tile(
                    [P, HG * D], bf16, tag=f"wv{hg % 2}_{k}",
                    name=f"wv{hg}_{k}"
                )
                nc.gpsimd.dma_start(
                    out=wt, in_=wv_t[k][:, d0 : d0 + HG * D]
                )
                wvl.append(wt)
            if hg == 0:
                for k in range(NB_C):
                    nc.gpsimd.dma_start(out=sb_w1[k], in_=w1_t[k])
                for m in range(NB_F):
                    nc.gpsimd.dma_start(out=sb_w2[m], in_=w2_t[m])
            v4 = []
            for sc in range(T // P):
                vt = p_v.tile(
                    [P, HG, 65], bf16, tag=f"v4_{sc}", name=f"v4_{sc}"
                )
                nc.vector.memset(vt[:, :, 64:65], 1.0)
                v4.append(vt)
            for sc in range(T // P):
                ps = p_ps.tile(
                    [P, HG * D], f32, tag="ps", bufs=2, name="ps_v"
                )
                for k in range(NB_C):
                    nc.tensor.matmul(
                        ps,
                        sb_xt[k][:, sc * P : (sc + 1) * P],
                        wvl[k],
                        start=(k == 0),
                        stop=(k == NB_C - 1),
                    )
                nc.scalar.copy(
                    v4[sc][:, :, 0:64],
                    ps[:].rearrange("p (h d) -> p h d", h=HG),
                )
                if sc >= 1:
                    norm_tick()

            nc.leave_named_scope(f"proj{hg}", _sc_p[0], False)

            # queue next group's q/k weight loads; its projection matmuls
            # interleave into this group's attention below
            if hg + 1 < N_HG:
                issue_qk_dmas(hg + 1, nc.gpsimd)
                gen = qk_units(hg + 1)
            else:
                gen = None

            _sc_a = nc.enter_named_scope(f"attn{hg}", False)

            # ---- attention: scores one s-chunk ahead of AV ----
            av_cur = {}
            prev = None   # (tb, sc, nsc, {pc: et}, c0)

            def emit_scores(tb, sc, pcs=pcs):
                nsc = 8 * tb + 8
                c0 = max(0, 64 * sc - 512 * tb)
                ets = {}
                for pc in pcs:
                    ps = p_ps.tile(
                        [P, 2, 512], f32, tag="ps", bufs=2, name="ps_sc"
                    )
                    for par in range(2):
                        nc.tensor.matmul(
                            ps[:, par, c0:512],
                            q_t[pc][par * 64 : par * 64 + 64,
                                    sc * P : (sc + 1) * P],
                            k_t[pc][par * 64 : par * 64 + 64,
                                    tb * 512 + c0 : (tb + 1) * 512],
                            start=True,
                            stop=True,
                        )
                    et = p_e.tile(
                        [P, 2, 512], bf16, tag="exp", bufs=4, name="et"
                    )
                    nc.scalar.activation(
                        out=et[:, :, c0:512],
                        in_=ps[:, :, c0:512],
                        func=AF.Exp,
                        scale=SCALE,
                    )
                    if sc >= 8 * tb:   # causal boundary stripe
                        nc.vector.tensor_tensor(
                            et[:, :, c0 : c0 + 64],
                            et[:, :, c0 : c0 + 64],
                            msk2[:, :, 0:64],
                            ALU.mult,
                        )
                    ets[pc] = et
                return (tb, sc, nsc, ets, c0)

            def emit_av(unit, pcs=pcs, heads=heads, v4=v4):
                tb, sc, nsc, ets, c0 = unit
                if sc == 0:
                    for h in heads:
                        av_cur[h] = p_ps.tile(
                            [P, 512], f32, tag=f"av{h % HG}",
                            name=f"av{h}"
                        )
                for pc in pcs:
                    for par in range(2):
                        h = 2 * pc + par
                        jj = heads.index(h)
                        nc.tensor.matmul(
                            av_cur[h][0:65, c0:512],
                            v4[sc][:, jj, :],
                            ets[pc][:, par, c0:512],
                            start=(sc == 0),
                            stop=(sc == nsc - 1),
                        )

            for tb in range(2):
                nsc = 8 * tb + 8
                for sc in range(nsc):
                    unit = emit_scores(tb, sc)
                    if gen is not None:
                        u = next(gen, None)
                        if u is None:
                            gen = None
                        else:
                            u()
                    if prev is not None:
                        emit_av(prev)
                        if prev[0] != tb:
                            # previous token-block fully accumulated:
                            # free its av banks with cheap copies; the
                            # reciprocal chain is deferred
                            ec, er, ebm = make_norm(prev[0], pcs, dict(av_cur))
                            ec()
                            queue_norm(er, ebm)
                    norm_tick()
                    prev = unit
            emit_av(prev)
            ec, er, ebm = make_norm(1, pcs, dict(av_cur))
            ec()
            queue_norm(er, ebm)
            if gen is not None:
                for u in gen:
                    u()

            nc.leave_named_scope(f"attn{hg}", _sc_a[0], False)

        while pending_norm:
            pending_norm.pop(0)[1]()

        attn_stack.close()   # free attention PSUM + den tiles
        xt_stack.close()     # free xT before the MLP pools open

        # ============================================================
        # Phase B: residual + LN1 + MLP + residual + LN2, per tb
        # ============================================================
        with contextlib.ExitStack() as mctx:
            mctx.enter_context(nc.named_scope("mlp"))
            p_r1 = mctx.enter_context(tc.tile_pool(name="r1", bufs=1))
            p_ln = mctx.enter_context(tc.tile_pool(name="ln", bufs=1))
            p_tmp = mctx.enter_context(tc.tile_pool(name="tmp", bufs=1))
            p_st = mctx.enter_context(tc.tile_pool(name="st", bufs=1))
            p_psm = mctx.enter_context(
                tc.tile_pool(name="psm", bufs=1, space="PSUM")
            )
            p_h = mctx.enter_context(tc.tile_pool(name="hsb", bufs=1))
            p_out = mctx.enter_context(tc.tile_pool(name="outp", bufs=1))

            r1 = [
                p_r1.tile([P, OT], bf16, tag=f"r1_{c}", name=f"r1_{c}")
                for c in range(NB_C)
            ]
            ln1 = [
                p_ln.tile([P, OT], bf16, tag=f"ln1_{c}", name=f"ln1_{c}")
                for c in range(NB_C)
            ]

            def layer_norm_T(src_tiles, gt, bt, out_cb):
                """transposed LN over the partition (c) dim. src_tiles are
                bf16 [128, 512] SBUF views. Emits:
                  stats:  mu_ps = sum_c src ; sq_ps = sum_c Square(src)
                  A = 1 (x) rsig, B = 1 (x) mu*rsig  (PSUM, rank-1 PE bcast)
                  out_cb(c, t, B_ps) with t = src*A  (so out = t - B [*g+b])
                """
                mu_ps = p_psm.tile([1, 512], f32, tag="lnst", bufs=2, name="mu_ps")
                sq_ps = p_psm.tile([1, 512], f32, tag="lnst", bufs=2, name="sq_ps")
                for c in range(NB_C):
                    s = p_tmp.tile([P, 512], bf16, tag="sqt", bufs=2, name="sqt")
                    nc.scalar.activation(out=s, in_=src_tiles[c], func=AF.Square)
                    nc.tensor.matmul(
                        mu_ps, ones_k, src_tiles[c],
                        start=(c == 0), stop=(c == NB_C - 1),
                    )
                    nc.tensor.matmul(
                        sq_ps, ones_k, s,
                        start=(c == 0), stop=(c == NB_C - 1),
                    )
                mu = p_st.tile([1, 512], f32, tag="mu_s", bufs=2, name="mu")
                nc.vector.tensor_scalar_mul(mu, mu_ps, 1.0 / C)
                mumu = p_st.tile([1, 512], f32, tag="mumu", bufs=2, name="mumu")
                nc.vector.tensor_tensor(mumu, mu, mu, ALU.mult)
                var = p_st.tile([1, 512], f32, tag="var", bufs=2, name="var")
                nc.vector.scalar_tensor_tensor(
                    var, sq_ps, 1.0 / C, mumu, ALU.mult, ALU.subtract
                )
                # rsqrt(var+eps) = exp(-0.5*ln(var+eps)): ln/exp share one
                # activation table (no swaps vs sqrt + reciprocal)
                lnv = p_st.tile([1, 512], f32, tag="lnv", bufs=2, name="lnv")
                nc.scalar.activation(
                    out=lnv, in_=var, func=AF.Ln, bias=eps_t, scale=1.0
                )
                rsg = p_st.tile([1, 512], bf16, tag="rsg", bufs=2, name="rsg")
                nc.scalar.activation(
                    out=rsg, in_=lnv, func=AF.Exp, scale=-0.5
                )
                m2 = p_st.tile([1, 512], bf16, tag="m2", bufs=2, name="m2")
                nc.vector.tensor_tensor(m2, mu, rsg, ALU.mult)
                a_ps = p_psm.tile([P, 512], f32, tag="lnbc", bufs=2, name="a_ps")
                nc.tensor.matmul(a_ps, ones_row, rsg, start=True, stop=True)
                b_ps = p_psm.tile([P, 512], f32, tag="lnbc", bufs=2, name="b_ps")
                nc.tensor.matmul(b_ps, ones_row, m2, start=True, stop=True)
                for c in range(NB_C):
                    t = p_tmp.tile([P, 512], f32, tag="d1", bufs=2, name="d1")
                    nc.vector.tensor_tensor(t, src_tiles[c], a_ps, ALU.mult)
                    out_cb(c, t, b_ps, gt, bt)

            # residual + LN1 for BOTH halves first (tb1's DVE work then
            # overlaps MLP(tb0)'s matmuls)
            for tb in range(2):
                sl = slice(tb * 512, (tb + 1) * 512)
                r1v = []
                for c in range(NB_C):
                    nc.vector.tensor_tensor(
                        r1[c][:, sl], sb_xto16[c][:, sl], sb_a[c][:, sl],
                        ALU.add,
                    )
                    r1v.append(r1[c][:, sl])

                if trivial_gb:
                    def ln1_out(c, t, b_ps, gt, bt, _sl=sl):
                        nc.vector.tensor_tensor(
                            ln1[c][:, _sl], t, b_ps, ALU.subtract
                        )
                else:
                    def ln1_out(c, t, b_ps, gt, bt, _sl=sl):
                        d = p_tmp.tile([P, 512], f32, tag="d2", bufs=2, name="d2")
                        nc.vector.tensor_tensor(d, t, b_ps, ALU.subtract)
                        nc.vector.tensor_scalar(
                            out=ln1[c][:, _sl], in0=d,
                            scalar1=gt[:, c : c + 1], scalar2=bt[:, c : c + 1],
                            op0=ALU.mult, op1=ALU.add,
                        )

                layer_norm_T(r1v, sb_g1, sb_be1, ln1_out)

            def mlp_pass1(tb):
                sl = slice(tb * 512, (tb + 1) * 512)
                h_sb = []
                for m in range(NB_F):
                    h_ps = p_psm.tile([P, 512], f32, tag="h_ps", bufs=2,
                                      name="h_ps")
                    for k in range(NB_C):
                        nc.tensor.matmul(
                            h_ps,
                            sb_w1[k][:, m * P : (m + 1) * P],
                            ln1[k][:, sl],
                            start=(k == 0),
                            stop=(k == NB_C - 1),
                        )
                    hs = p_h.tile([P, 512], bf16, tag=f"h{m}", name=f"h{m}")
                    nc.scalar.activation(
                        out=hs, in_=h_ps, func=AF.Gelu,
                        bias=sb_b1[:, m : m + 1], scale=1.0,
                    )
                    h_sb.append(hs)
                return h_sb

            def mlp_pass2(tb, h_sb):
                # r2 = W2.T h + b2 + ln1  (single fused DVE op per chunk;
                # result overwrites r1, bf16)
                sl = slice(tb * 512, (tb + 1) * 512)
                r2v = []
                for c in range(NB_C):
                    y_ps = p_psm.tile([P, 512], f32, tag="y_ps", bufs=2,
                                      name="y_ps")
                    for m in range(NB_F):
                        nc.tensor.matmul(
                            y_ps,
                            sb_w2[m][:, c * P : (c + 1) * P],
                            h_sb[m],
                            start=(m == 0),
                            stop=(m == NB_F - 1),
                        )
                    nc.vector.scalar_tensor_tensor(
                        r1[c][:, sl], y_ps, sb_b2[:, c : c + 1],
                        ln1[c][:, sl], ALU.add, ALU.add,
                    )
                    r2v.append(r1[c][:, sl])
                return r2v

            def ln2_emit(tb, r2v):
                sl = slice(tb * 512, (tb + 1) * 512)

                if trivial_gb:
                    def ln2_out(c, t, b_ps, gt, bt, _sl=sl):
                        o = p_out.tile([P, 512], f32, tag="o", bufs=2, name="o")
                        nc.vector.tensor_tensor(o, t, b_ps, ALU.subtract)
                        nc.gpsimd.dma_start(out=outT_t[c][:, _sl], in_=o)
                else:
                    def ln2_out(c, t, b_ps, gt, bt, _sl=sl):
                        d = p_tmp.tile([P, 512], f32, tag="d2", bufs=2, name="d2")
                        nc.vector.tensor_tensor(d, t, b_ps, ALU.subtract)
                        o = p_out.tile([P, 512], f32, tag="o", bufs=2, name="o")
                        nc.vector.tensor_scalar(
                            out=o, in0=d,
                            scalar1=gt[:, c : c + 1], scalar2=bt[:, c : c + 1],
                            op0=ALU.mult, op1=ALU.add,
                        )
                        nc.gpsimd.dma_start(out=outT_t[c][:, _sl], in_=o)

                layer_norm_T(r2v, sb_g2, sb_be2, ln2_out)

            # order chosen to keep the PE stream dense and minimize ACT
            # table swaps: M1(0) M2(0) M1(1) LN2(0) M2(1) LN2(1)
            h0 = mlp_pass1(0)
            r2v0 = mlp_pass2(0, h0)
            h1 = mlp_pass1(1)
            ln2_emit(0, r2v0)
            r2v1 = mlp_pass2(1, h1)
            ln2_emit(1, r2v1)

    return nc


def _spill_excess_waits(nc, maxw=2):
    """walrus (this build) caps sync-wait commands per instruction. Move
    excess waits onto freshly inserted same-engine nops placed immediately
    before the over-limit instruction (same engine stream => the waits
    still complete before it executes)."""
    import copy

    import concourse.bass as bass
    import concourse.mybir as mybir

    scratch = bass.Bass()
    tpl = scratch.sync.nop(nofuse=True).ins
    ctr = [0]

    def mknop(engine, waits):
        n = copy.deepcopy(tpl)
        ctr[0] += 1
        n.name = f"I-spill{ctr[0]}"
        n.engine = engine
        n.sync_info = mybir.SyncInfo(on_wait=list(waits), on_update=[])
        return n

    fn = nc.m.functions[0]
    for bb in fn.blocks:
        changed = False
        out = []
        for inst in bb.instructions:
            si = inst.sync_info
            waits = list(si.on_wait) if si and si.on_wait else []
            nupd = len(si.on_update) if si and si.on_update else 0
            lim = max(0, maxw - nupd)   # waits + updates <= maxw total
            if len(waits) > lim:
                keep = waits[-lim:] if lim else []
                rest = waits[: len(waits) - lim]
                while rest:
                    chunk, rest = rest[:1], rest[1:]
                    out.append(mknop(inst.engine, chunk))
                si.on_wait = keep
                changed = True
            out.append(inst)
        if changed:
            bb.instructions = out
    return nc


def _get_nc(trivial_gb):
    key = ("nc", trivial_gb)
    if key not in _compiled:
        _patch_tile_drain()
        _patch_profile_hook()
        nc = _build_nc(trivial_gb)
        _spill_excess_waits(nc, maxw=2)
        _compiled[key] = nc
    return _compiled[key]


# --------------------------------------------------------------------------
# host-side sharding
# --------------------------------------------------------------------------

def _make_in_maps(x, Wq, Wk, Wv, ln1_g, ln1_b, W1, b1, W2, b2, ln2_g, ln2_b):
    x = np.asarray(x, np.float32)
    wq_s = np.ascontiguousarray(
        np.asarray(Wq, np.float32).transpose(1, 0, 2).reshape(C, C)
    ).astype(BF16)
    wk_s = np.ascontiguousarray(
        np.asarray(Wk, np.float32).transpose(1, 0, 2).reshape(C, C)
    ).astype(BF16)
    wv_s = np.ascontiguousarray(
        np.asarray(Wv, np.float32).transpose(1, 0, 2).reshape(C, C)
    ).astype(BF16)
    w1b = np.asarray(W1, np.float32).astype(BF16)
    w2b = np.asarray(W2, np.float32).astype(BF16)
    b1r = np.ascontiguousarray(np.asarray(b1, np.float32).reshape(NB_F, P).T)
    b2r = np.ascontiguousarray(np.asarray(b2, np.float32).reshape(NB_C, P).T)
    g1r = np.ascontiguousarray(np.asarray(ln1_g, np.float32).reshape(NB_C, P).T)
    be1r = np.ascontiguousarray(np.asarray(ln1_b, np.float32).reshape(NB_C, P).T)
    g2r = np.ascontiguousarray(np.asarray(ln2_g, np.float32).reshape(NB_C, P).T)
    be2r = np.ascontiguousarray(np.asarray(ln2_b, np.float32).reshape(NB_C, P).T)

    in_maps = []
    for core in range(N_CORES):
        b, g = core // 2, core % 2
        xb = x[b]                                # [T, C]
        xTa = np.ascontiguousarray(xb.T)         # [C, T]
        own = np.arange(g, T, 2)
        xo = np.ascontiguousarray(xb[own].T)     # [C, OT]
        ii = np.arange(P)[:, None]
        mm = np.arange(64)[None, :]
        cm = np.where(ii <= 2 * mm + g, 1.0, 0.0).astype(BF16)
        in_maps.append(
            {
                "xT": xTa.astype(BF16),
                "xTo16": xo.astype(BF16),
                "wq": wq_s,
                "wk": wk_s,
                "wv": wv_s,
                "w1": w1b,
                "w2": w2b,
                "b1r": b1r,
                "b2r": b2r,
                "g1r": g1r,
                "be1r": be1r,
                "g2r": g2r,
                "be2r": be2r,
                "cmask": cm,
            }
        )
    return in_maps


def _assemble(results):
    out = np.empty((B, T, C), np.float32)
    for core in range(N_CORES):
        b, g = core // 2, core % 2
        own = np.arange(g, T, 2)
        out[b, own, :] = results[core]["outT"].T
    return out


def kernel(_trace=False, **inputs):
    from concourse.bass_utils import run_bass_kernel_spmd

    trivial_gb = bool(
        np.all(np.asarray(inputs["ln1_g"]) == 1.0)
        and np.all(np.asarray(inputs["ln1_b"]) == 0.0)
        and np.all(np.asarray(inputs["ln2_g"]) == 1.0)
        and np.all(np.asarray(inputs["ln2_b"]) == 0.0)
    )
    nc = _get_nc(trivial_gb)
    in_maps = _make_in_maps(**inputs)
    res = run_bass_kernel_spmd(nc, in_maps, list(range(N_CORES)), trace=_trace)
    out = _assemble(res.results)
    if _trace:
        return out, res
    return out


# revision 23
# speedup vs baseline: 1.2325x; 1.2325x over previous
"""Trainium2 Bass kernel for a single transformer block (nn_Block_3212635537783).

Reference computation (B=4, T=2048, C=768, H=12, D=64):
    q/k/v per-head projections of x; scores[t,s] = k[t]@q[s]/sqrt(C) with
    causal mask (s <= t), softmax over s; a[t] = sum_s w[t,s] v[s];
    x = LN1(x + a); x = LN2(x + gelu(x@W1 + b1)@W2 + b2)

Sharding: 8 cores = 4 batches x 2 token-interleaved halves. Core (b, g)
owns rows {g, g+2, ...} of batch b. The stride-2 interleave keeps the
causal workload balanced AND the SPMD program identical across cores
(only input data differs; the +-1 row causal boundary lives in a tiny
per-core mask tile).

On-chip layout is fully "transposed": activations are [C, tokens]
(feature dim on partitions) so attention, layernorm and the MLP never
need an on-chip transpose. Matmul inputs are bf16 (fp32 accumulation).

v2 scheduling notes:
 - AV matmuls are issued one s-chunk behind the score matmuls so the PE
   never waits in-line on the ACT exp of the same chunk.
 - softmax denominators: 1/den = exp(-ln(den)) on ACT (ln shares the
   exp activation table: zero table swaps) -> rank-1 PE broadcast into
   the unused upper rows of the av PSUM bank -> DVE stage + multiply;
   the chain is emitted *after* the next token-block's first scores so
   its latency hides under real PE work.
 - layernorm: bf16 residual stream; stats via ones-matmuls (Square on
   ACT); rsqrt = exp(-0.5*ln(var+eps)) on ACT; (x*A - B) form where
   A = 1 (x) rs, B = 1 (x) mu*rs ride two rank-1 PE broadcasts read
   directly from PSUM by the DVE (no ACT staging copies).
 - all HBM loads are issued from the (otherwise idle) GpSimd sequencer
   (25ns dispatch vs 565ns on Sync), most-urgent tiles first.
"""

import sys
import types

import numpy as np
import ml_dtypes

B, T, C, H, D = 4, 2048, 768, 12, 64
F = 4 * C            # 3072
P = 128              # partitions
OT = T // 2          # owned tokens per core (1024)
NB_C = C // P        # 6 c-chunks
NB_F = F // P        # 24 hidden chunks
NPAIR = H // 2       # 6 head-pair chunks
EPS = 1e-5
SCALE = float(1.0 / np.sqrt(np.float32(C)))
N_CORES = 8
HG = 4               # heads per attention group
N_HG = H // HG       # 3 groups

BF16 = ml_dtypes.bfloat16

_compiled = {}


# --------------------------------------------------------------------------
# environment patches (must live in kernel.py: the grader imports only this
# file). Idempotent.
# --------------------------------------------------------------------------

def _patch_tile_drain():
    """This walrus build rejects >1 sync-wait command on the final Tile
    drain CTRL instruction. Spread the drain's waits across chained
    sync-engine nops (same engine => program order preserved; the
    all-engine barrier after them still gates the semaphore clears)."""
    import concourse.tile as tile_mod
    import concourse.mybir as mybir

    if getattr(tile_mod.TileContext, "_drain_patched", False):
        return

    def patched(self, tick_clock, wait_clock):
        from concourse.vector_clock import ScopedClock

        drain_inst = self.nc.sync.drain()
        wait_clock.add_sem_waits(
            drain_inst.ins, ScopedClock({None: tick_clock.global_clock})
        )
        si = drain_inst.ins.sync_info
        waits = list(si.on_wait) if si else []
        MAXW = 1
        if len(waits) > MAXW:
            si.on_wait = waits[:MAXW]
            rest = waits[MAXW:]
            while rest:
                nop = self.nc.sync.nop(nofuse=True)
                chunk, rest = rest[:MAXW], rest[MAXW:]
                nsi = nop.ins.sync_info
                if nsi is None:
                    nop.ins.sync_info = mybir.SyncInfo(on_wait=chunk, on_update=[])
                else:
                    nsi.on_wait = list(nsi.on_wait) + chunk
        self.nc.all_engine_barrier()
        assert self.sems is not None
        popped = self.nc._tile_sem_poison_stack.pop()
        assert popped is self._sem_poison
        self.nc.clear_and_free_semaphores(list(self.sems.allocated().values()))
        self.nc.all_engine_barrier()

    tile_mod.TileContext._drain_and_barrier = patched
    tile_mod.TileContext._drain_patched = True


def _patch_profile_hook():
    """Optional: register the axon NTFF profiling hook so trace=True works
    (used for timing; harmless no-op if unavailable)."""
    if "antenv.axon_hooks" in sys.modules:
        return
    try:
        sys.path.insert(0, "/root/.axon_site")
        from trn_agent_boot.trn_boot import _ntff_profile_via_ctypes

        hook = _ntff_profile_via_ctypes("/opt/axon/libaxon_pjrt.so")
        mod = types.ModuleType("antenv.axon_hooks")
        mod.get_axon_ntff_profile_hook = lambda: hook
        mod.set_axon_ntff_profile_hook = lambda h: None
        sys.modules["antenv.axon_hooks"] = mod
        import concourse.bass_utils as bu

        bu.upload_artifacts = lambda tmpdir: "local://" + tmpdir
    except Exception:
        pass


# --------------------------------------------------------------------------
# program construction (shared by all 8 cores; SPMD over input data)
# --------------------------------------------------------------------------

def _build_nc(trivial_gb):
    import contextlib

    import concourse.bass as bass
    import concourse.mybir as mybir
    from concourse.tile import TileContext

    f32 = mybir.dt.float32
    f32r = mybir.dt.float32r
    bf16 = mybir.dt.bfloat16
    ALU = mybir.AluOpType
    AF = mybir.ActivationFunctionType

    nc = bass.Bass()

    # ---- DRAM I/O ----
    xT = nc.declare_dram_parameter("xT", [C, T], bf16, isOutput=False)
    xTo16 = nc.declare_dram_parameter("xTo16", [C, OT], bf16, isOutput=False)
    wq = nc.declare_dram_parameter("wq", [C, C], bf16, isOutput=False)
    wk = nc.declare_dram_parameter("wk", [C, C], bf16, isOutput=False)
    wv = nc.declare_dram_parameter("wv", [C, C], bf16, isOutput=False)
    w1 = nc.declare_dram_parameter("w1", [C, F], bf16, isOutput=False)
    w2 = nc.declare_dram_parameter("w2", [F, C], bf16, isOutput=False)
    b1r = nc.declare_dram_parameter("b1r", [P, NB_F], f32, isOutput=False)
    b2r = nc.declare_dram_parameter("b2r", [P, NB_C], f32, isOutput=False)
    g1r = nc.declare_dram_parameter("g1r", [P, NB_C], f32, isOutput=False)
    be1r = nc.declare_dram_parameter("be1r", [P, NB_C], f32, isOutput=False)
    g2r = nc.declare_dram_parameter("g2r", [P, NB_C], f32, isOutput=False)
    be2r = nc.declare_dram_parameter("be2r", [P, NB_C], f32, isOutput=False)
    cmask = nc.declare_dram_parameter("cmask", [P, 64], bf16, isOutput=False)
    outT = nc.declare_dram_parameter("outT", [C, OT], f32, isOutput=True)

    xT_t = xT[:].rearrange("(n p) t -> n p t", p=P)
    xTo16_t = xTo16[:].rearrange("(n p) t -> n p t", p=P)
    wq_t = wq[:].rearrange("(n p) c -> n p c", p=P)
    wk_t = wk[:].rearrange("(n p) c -> n p c", p=P)
    wv_t = wv[:].rearrange("(n p) c -> n p c", p=P)
    w1_t = w1[:].rearrange("(n p) f -> n p f", p=P)
    w2_t = w2[:].rearrange("(n p) c -> n p c", p=P)
    outT_t = outT[:].rearrange("(n p) t -> n p t", p=P)

    with TileContext(nc) as tc, contextlib.ExitStack() as ctx:
        const = ctx.enter_context(tc.tile_pool(name="const", bufs=1))
        p_a = ctx.enter_context(tc.tile_pool(name="attn_a", bufs=1))
        p_xo = ctx.enter_context(tc.tile_pool(name="xo", bufs=1))
        p_mlpw = ctx.enter_context(tc.tile_pool(name="mlpw", bufs=1))
        import contextlib as _ctl
        xt_stack = _ctl.ExitStack()
        p_xt = xt_stack.enter_context(tc.tile_pool(name="xt", bufs=1))

        # ---- constants (memsets now; const DMAs issued later on gpsimd
        # so the critical first loads go out first) ----
        ones_k = const.tile([P, 1], bf16, tag="ones_k", name="ones_k")
        nc.vector.memset(ones_k, 1.0)
        ones_row = const.tile([1, P], bf16, tag="ones_row", name="ones_row")
        nc.vector.memset(ones_row, 1.0)
        eps_t = const.tile([1, 1], f32, tag="eps", name="eps_t")
        nc.vector.memset(eps_t, EPS)
        msk = const.tile([P, 64], bf16, tag="msk", name="msk")
        msk2 = bass.AP(
            tensor=msk.tensor, offset=msk.offset,
            ap=[list(msk.ap[0]), [0, 2], list(msk.ap[1])],
        )
        sb_b1 = const.tile([P, NB_F], f32, tag="b1", name="sb_b1")
        sb_b2 = const.tile([P, NB_C], f32, tag="b2", name="sb_b2")
        sb_g1 = const.tile([P, NB_C], f32, tag="g1", name="sb_g1")
        sb_be1 = const.tile([P, NB_C], f32, tag="be1", name="sb_be1")
        sb_g2 = const.tile([P, NB_C], f32, tag="g2", name="sb_g2")
        sb_be2 = const.tile([P, NB_C], f32, tag="be2", name="sb_be2")

        # ---- persistent activation tiles ----
        sb_xt = [
            p_xt.tile([P, T], bf16, tag=f"xt{k}", name=f"xt{k}")
            for k in range(NB_C)
        ]
        sb_xto16 = [
            p_xo.tile([P, OT], bf16, tag=f"xto16_{k}", name=f"xto16_{k}")
            for k in range(NB_C)
        ]

        # attention output a^T, bf16 [128, OT] per pair-chunk
        sb_a = [
            p_a.tile([P, OT], bf16, tag=f"a{pc}", name=f"a{pc}")
            for pc in range(NPAIR)
        ]

        # MLP weight tiles (DMA'd later, after attention weights)
        sb_w1 = [
            p_mlpw.tile([P, F], bf16, tag=f"w1_{k}", name=f"w1_{k}")
            for k in range(NB_C)
        ]
        sb_w2 = [
            p_mlpw.tile([P, C], bf16, tag=f"w2_{m}", name=f"w2_{m}")
            for m in range(NB_F)
        ]

        # ============================================================
        # Phase A: attention, in head groups of HG. All attention pools
        # are global (slot-ring tags) so projection work for group hg+1
        # can interleave into attention of group hg.
        # ============================================================
        attn_stack = _ctl.ExitStack()
        p_ps = attn_stack.enter_context(
            tc.tile_pool(name="attnps", bufs=1, space="PSUM")
        )
        p_dn = attn_stack.enter_context(tc.tile_pool(name="dn", bufs=1))
        p_w = attn_stack.enter_context(tc.tile_pool(name="wqk", bufs=1))
        p_qk = attn_stack.enter_context(tc.tile_pool(name="qk", bufs=1))
        p_v = attn_stack.enter_context(tc.tile_pool(name="vv", bufs=1))
        p_e = attn_stack.enter_context(tc.tile_pool(name="ee", bufs=1))

        def group_pcs(hg):
            return [hg * (HG // 2) + i for i in range(HG // 2)]

        wq_sb, wk_sb, q_t, k_t = {}, {}, {}, {}

        def issue_qk_dmas(hg, eng):
            """Allocate weight/output tiles for group hg's q/k projections
            and queue the weight loads on `eng`'s sequencer."""
            for pc in group_pcs(hg):
                wq_sb[pc] = []
                for k in range(NB_C):
                    wt = p_w.tile(
                        [P, P], bf16, tag=f"wq{pc % 4}_{k}",
                        name=f"wq{pc}_{k}"
                    )
                    eng.dma_start(
                        out=wt, in_=wq_t[k][:, pc * P : (pc + 1) * P]
                    )
                    wq_sb[pc].append(wt)
            for pc in group_pcs(hg):
                wk_sb[pc] = []
                for k in range(NB_C):
                    wt = p_w.tile(
                        [P, P], bf16, tag=f"wk{pc % 4}_{k}",
                        name=f"wk{pc}_{k}"
                    )
                    eng.dma_start(
                        out=wt, in_=wk_t[k][:, pc * P : (pc + 1) * P]
                    )
                    wk_sb[pc].append(wt)

        def qk_units(hg):
            """Generator: one closure per projection psum-tile for group
            hg's q/k projections (8 q units + 4 k units)."""
            for pc in group_pcs(hg):
                q_t[pc] = p_qk.tile(
                    [P, T], bf16, tag=f"q{pc % 4}", name=f"q{pc}"
                )
                k_t[pc] = p_qk.tile(
                    [P, OT], bf16, tag=f"k{pc % 4}", name=f"k{pc}"
                )
            for pc in group_pcs(hg):
                for t4 in range(T // 512):
                    def qu(pc=pc, t4=t4):
                        ps = p_ps.tile(
                            [P, 512], f32, tag="ps", bufs=2, name="ps_prj"
                        )
                        for k in range(NB_C):
                            nc.tensor.matmul(
                                ps,
                                wq_sb[pc][k],
                                sb_xt[k][:, t4 * 512 : (t4 + 1) * 512],
                                start=(k == 0),
                                stop=(k == NB_C - 1),
                            )
                        nc.vector.tensor_copy(
                            q_t[pc][:, t4 * 512 : (t4 + 1) * 512], ps
                        )
                    yield qu
                for t2 in range(OT // 512):
                    def ku(pc=pc, t2=t2):
                        ps = p_ps.tile(
                            [P, 512], f32, tag="ps", bufs=2, name="ps_prk"
                        )
                        for k in range(NB_C):
                            nc.tensor.matmul(
                                ps,
                                wk_sb[pc][k],
                                sb_xto16[k][:, t2 * 512 : (t2 + 1) * 512],
                                start=(k == 0),
                                stop=(k == NB_C - 1),
                            )
                        nc.vector.tensor_copy(
                            k_t[pc][:, t2 * 512 : (t2 + 1) * 512], ps
                        )
                    yield ku

        # ---- DMA issue: critical path (group-0 q weights + first xT
        # quarter) on the Sync sequencer; everything else on GpSimd in
        # urgency order. Both sequencers issue in parallel. ----
        issue_qk_dmas(0, nc.sync)
        for k in range(NB_C):
            nc.sync.dma_start(out=sb_xt[k][:, 0:512], in_=xT_t[k][:, 0:512])
        for k in range(NB_C):
            nc.gpsimd.dma_start(out=sb_xto16[k], in_=xTo16_t[k])
        for quarter in range(1, 4):
            cs = slice(quarter * 512, (quarter + 1) * 512)
            for k in range(NB_C):
                nc.gpsimd.dma_start(out=sb_xt[k][:, cs], in_=xT_t[k][:, cs])
        nc.gpsimd.dma_start(out=msk, in_=cmask[:])
        nc.gpsimd.dma_start(out=sb_b1, in_=b1r[:])
        nc.gpsimd.dma_start(out=sb_b2, in_=b2r[:])
        nc.gpsimd.dma_start(out=sb_g1, in_=g1r[:])
        nc.gpsimd.dma_start(out=sb_be1, in_=be1r[:])
        nc.gpsimd.dma_start(out=sb_g2, in_=g2r[:])
        nc.gpsimd.dma_start(out=sb_be2, in_=be2r[:])

        def make_norm(tb, pcs, av):
            """Softmax normalization for one token-block. emit_copy frees
            the av PSUM banks with 4 cheap DVE copies (all the new block's
            AV accumulation waits on); the reciprocal chain in emit_rest
            runs entirely off the critical path against the copies."""
            av_sb = {}
            heads = [2 * pc + j for pc in pcs for j in range(2)]

            def emit_copy():
                for h in heads:
                    t = p_dn.tile([65, 512], bf16, tag="avsb", bufs=8,
                                  name="av_sb")
                    nc.vector.tensor_copy(t, av[h][0:65, 0:512])
                    av_sb[h] = t

            recs = {}

            def emit_recs():
                # 1/den = exp(-ln(den)) on ACT (ln shares exp's table)
                for h in heads:
                    lnd = p_dn.tile([1, 512], f32, tag="lnd", bufs=4,
                                    name="lnd")
                    nc.scalar.activation(
                        out=lnd, in_=av_sb[h][64:65, :], func=AF.Ln
                    )
                    rec = p_dn.tile([1, 512], bf16, tag="rec", bufs=4,
                                    name="rec")
                    nc.scalar.activation(
                        out=rec, in_=lnd, func=AF.Exp, scale=-1.0
                    )
                    recs[h] = rec

            def emit_bm():
                # rank-1 broadcast of 1/den + normalize multiply. Emitted
                # well after emit_recs so the borrowed score-ring PSUM
                # slot is held only briefly.
                dps = p_ps.tile([P, 2, 512], f32, tag="ps", bufs=2,
                                name="den_ps")
                for j, pc in enumerate(pcs):
                    for par in range(2):
                        nc.tensor.matmul(
                            dps[par * 64 : par * 64 + 64, j, :],
                            ones_row[:, 0:64],
                            recs[2 * pc + par],
                            start=True, stop=True,
                        )
                for j, pc in enumerate(pcs):
                    for par in range(2):
                        h = 2 * pc + par
                        nc.vector.tensor_tensor(
                            sb_a[pc][par * 64 : par * 64 + 64,
                                     tb * 512 : (tb + 1) * 512],
                            av_sb[h][0:64, :],
                            dps[par * 64 : par * 64 + 64, j, :],
                            ALU.mult,
                        )

            return emit_copy, emit_recs, emit_bm

        pending_norm = []   # (due_tick, closure), ascending
        norm_clock = [0]

        def queue_norm(emit_recs, emit_bm):
            pending_norm.append((norm_clock[0] + 2, emit_recs))
            pending_norm.append((norm_clock[0] + 6, emit_bm))

        def norm_tick():
            norm_clock[0] += 1
            while pending_norm and pending_norm[0][0] <= norm_clock[0]:
                pending_norm.pop(0)[1]()

        for hg in range(N_HG):
            pcs = group_pcs(hg)
            heads = [2 * pc + j for pc in pcs for j in range(2)]

            _sc_p = nc.enter_named_scope(f"proj{hg}", False)
            if hg == 0:
                for u in qk_units(0):
                    u()

            # ---- v projection block for this group ----
            d0 = heads[0] * D
            wvl = []
            for k in range(NB_C):
                wt = p_w.tile(
                    [P, HG * D], bf16, tag=f"wv{hg % 2}_{k}",
                    name=f"wv{hg}_{k}"
                )
                nc.gpsimd.dma_start(
                    out=wt, in_=wv_t[k][:, d0 : d0 + HG * D]
                )
                wvl.append(wt)
            if hg == 0:
                for k in range(NB_C):
                    nc.gpsimd.dma_start(out=sb_w1[k], in_=w1_t[k])
                for m in range(NB_F):
                    nc.gpsimd.dma_start(out=sb_w2[m], in_=w2_t[m])
            v4 = []
            for sc in range(T // P):
                vt = p_v.tile(
                    [P, HG, 65], bf16, tag=f"v4_{sc}", name=f"v4_{sc}"
                )
                nc.vector.memset(vt[:, :, 64:65], 1.0)
                v4.append(vt)
            for sc in range(T // P):
                ps = p_ps.tile(
                    [P, HG * D], f32, tag="ps", bufs=2, name="ps_v"
                )
                for k in range(NB_C):
                    nc.tensor.matmul(
                        ps,
                        sb_xt[k][:, sc * P : (sc + 1) * P],
                        wvl[k],
                        start=(k == 0),
                        stop=(k == NB_C - 1),
                    )
                nc.scalar.copy(
                    v4[sc][:, :, 0:64],
                    ps[:].rearrange("p (h d) -> p h d", h=HG),
                )
                if sc >= 1:
                    norm_tick()

            nc.leave_named_scope(f"proj{hg}", _sc_p[0], False)

            # queue next group's q/k weight loads; its projection matmuls
            # interleave into this group's attention below
            if hg + 1 < N_HG:
                issue_qk_dmas(hg + 1, nc.gpsimd)
                gen = qk_units(hg + 1)
            else:
                gen = None

            _sc_a = nc.enter_named_scope(f"attn{hg}", False)

            # ---- attention: scores one s-chunk ahead of AV ----
            av_cur = {}
            prev = None   # (tb, sc, nsc, {pc: et}, c0)

            def emit_scores(tb, sc, pcs=pcs):
                nsc = 8 * tb + 8
                c0 = max(0, 64 * sc - 512 * tb)
                ets = {}
                for pc in pcs:
                    ps = p_ps.tile(
                        [P, 2, 512], f32, tag="ps", bufs=2, name="ps_sc"
                    )
                    for par in range(2):
                        nc.tensor.matmul(
                            ps[:, par, c0:512],
                            q_t[pc][par * 64 : par * 64 + 64,
                                    sc * P : (sc + 1) * P],
                            k_t[pc][par * 64 : par * 64 + 64,
                                    tb * 512 + c0 : (tb + 1) * 512],
                            start=True,
                            stop=True,
                        )
                    et = p_e.tile(
                        [P, 2, 512], bf16, tag="exp", bufs=4, name="et"
                    )
                    nc.scalar.activation(
                        out=et[:, :, c0:512],
                        in_=ps[:, :, c0:512],
                        func=AF.Exp,
                        scale=SCALE,
                    )
                    if sc >= 8 * tb:   # causal boundary stripe
                        nc.vector.tensor_tensor(
                            et[:, :, c0 : c0 + 64],
                            et[:, :, c0 : c0 + 64],
                            msk2[:, :, 0:64],
                            ALU.mult,
                        )
                    ets[pc] = et
                return (tb, sc, nsc, ets, c0)

            def emit_av(unit, pcs=pcs, heads=heads, v4=v4):
                tb, sc, nsc, ets, c0 = unit
                if sc == 0:
                    for h in heads:
                        av_cur[h] = p_ps.tile(
                            [P, 512], f32, tag=f"av{h % HG}",
                            name=f"av{h}"
                        )
                for pc in pcs:
                    for par in range(2):
                        h = 2 * pc + par
                        jj = heads.index(h)
                        nc.tensor.matmul(
                            av_cur[h][0:65, c0:512],
                            v4[sc][:, jj, :],
                            ets[pc][:, par, c0:512],
                            start=(sc == 0),
                            stop=(sc == nsc - 1),
                        )

            for tb in range(2):
                nsc = 8 * tb + 8
                for sc in range(nsc):
                    unit = emit_scores(tb, sc)
                    if prev is not None:
                        emit_av(prev)
                        if prev[0] != tb:
                            # previous token-block fully accumulated:
                            # free its av banks with cheap copies; the
                            # reciprocal chain is deferred
                            ec, er, ebm = make_norm(prev[0], pcs, dict(av_cur))
                            ec()
                            queue_norm(er, ebm)
                    norm_tick()
                    prev = unit
            emit_av(prev)
            ec, er, ebm = make_norm(1, pcs, dict(av_cur))
            ec()
            queue_norm(er, ebm)
            if gen is not None:
                for u in gen:
                    u()
                    norm_tick()

            nc.leave_named_scope(f"attn{hg}", _sc_a[0], False)

        while pending_norm:
            pending_norm.pop(0)[1]()

        attn_stack.close()   # free attention PSUM + den tiles
        xt_stack.close()     # free xT before the MLP pools open

        # ============================================================
        # Phase B: residual + LN1 + MLP + residual + LN2, per tb
        # ============================================================
        with contextlib.ExitStack() as mctx:
            mctx.enter_context(nc.named_scope("mlp"))
            p_r1 = mctx.enter_context(tc.tile_pool(name="r1", bufs=1))
            p_ln = mctx.enter_context(tc.tile_pool(name="ln", bufs=1))
            p_tmp = mctx.enter_context(tc.tile_pool(name="tmp", bufs=1))
            p_st = mctx.enter_context(tc.tile_pool(name="st", bufs=1))
            p_psm = mctx.enter_context(
                tc.tile_pool(name="psm", bufs=1, space="PSUM")
            )
            p_h = mctx.enter_context(tc.tile_pool(name="hsb", bufs=1))
            p_out = mctx.enter_context(tc.tile_pool(name="outp", bufs=1))

            r1 = [
                p_r1.tile([P, OT], bf16, tag=f"r1_{c}", name=f"r1_{c}")
                for c in range(NB_C)
            ]
            ln1 = [
                p_ln.tile([P, OT], bf16, tag=f"ln1_{c}", name=f"ln1_{c}")
                for c in range(NB_C)
            ]

            def layer_norm_T(src_tiles, gt, bt, out_cb):
                """transposed LN over the partition (c) dim. src_tiles are
                bf16 [128, 512] SBUF views. Emits:
                  stats:  mu_ps = sum_c src ; sq_ps = sum_c Square(src)
                  A = 1 (x) rsig, B = 1 (x) mu*rsig  (PSUM, rank-1 PE bcast)
                  out_cb(c, t, B_ps) with t = src*A  (so out = t - B [*g+b])
                """
                mu_ps = p_psm.tile([1, 512], f32, tag="lnst", bufs=2, name="mu_ps")
                sq_ps = p_psm.tile([1, 512], f32, tag="lnst", bufs=2, name="sq_ps")
                for c in range(NB_C):
                    s = p_tmp.tile([P, 512], bf16, tag="sqt", bufs=2, name="sqt")
                    nc.scalar.activation(out=s, in_=src_tiles[c], func=AF.Square)
                    nc.tensor.matmul(
                        mu_ps, ones_k, src_tiles[c],
                        start=(c == 0), stop=(c == NB_C - 1),
                    )
                    nc.tensor.matmul(
                        sq_ps, ones_k, s,
                        start=(c == 0), stop=(c == NB_C - 1),
                    )
                mu = p_st.tile([1, 512], f32, tag="mu_s", bufs=2, name="mu")
                nc.vector.tensor_scalar_mul(mu, mu_ps, 1.0 / C)
                mumu = p_st.tile([1, 512], f32, tag="mumu", bufs=2, name="mumu")
                nc.vector.tensor_tensor(mumu, mu, mu, ALU.mult)
                var = p_st.tile([1, 512], f32, tag="var", bufs=2, name="var")
                nc.vector.scalar_tensor_tensor(
                    var, sq_ps, 1.0 / C, mumu, ALU.mult, ALU.subtract
                )
                # rsqrt(var+eps) = exp(-0.5*ln(var+eps)): ln/exp share one
                # activation table (no swaps vs sqrt + reciprocal)
                lnv = p_st.tile([1, 512], f32, tag="lnv", bufs=2, name="lnv")
                nc.scalar.activation(
                    out=lnv, in_=var, func=AF.Ln, bias=eps_t, scale=1.0
                )
                rsg = p_st.tile([1, 512], bf16, tag="rsg", bufs=2, name="rsg")
                nc.scalar.activation(
                    out=rsg, in_=lnv, func=AF.Exp, scale=-0.5
                )
                m2 = p_st.tile([1, 512], bf16, tag="m2", bufs=2, name="m2")
                nc.vector.tensor_tensor(m2, mu, rsg, ALU.mult)
                a_ps = p_psm.tile([P, 512], f32, tag="lnbc", bufs=2, name="a_ps")
                nc.tensor.matmul(a_ps, ones_row, rsg, start=True, stop=True)
                b_ps = p_psm.tile([P, 512], f32, tag="lnbc", bufs=2, name="b_ps")
                nc.tensor.matmul(b_ps, ones_row, m2, start=True, stop=True)
                for c in range(NB_C):
                    t = p_tmp.tile([P, 512], f32, tag="d1", bufs=2, name="d1")
                    nc.vector.tensor_tensor(t, src_tiles[c], a_ps, ALU.mult)
                    out_cb(c, t, b_ps, gt, bt)

            # residual + LN1 for BOTH halves first (tb1's DVE work then
            # overlaps MLP(tb0)'s matmuls)
            for tb in range(2):
                sl = slice(tb * 512, (tb + 1) * 512)
                r1v = []
                for c in range(NB_C):
                    nc.vector.tensor_tensor(
                        r1[c][:, sl], sb_xto16[c][:, sl], sb_a[c][:, sl],
                        ALU.add,
                    )
                    r1v.append(r1[c][:, sl])

                if trivial_gb:
                    def ln1_out(c, t, b_ps, gt, bt, _sl=sl):
                        nc.vector.tensor_tensor(
                            ln1[c][:, _sl], t, b_ps, ALU.subtract
                        )
                else:
                    def ln1_out(c, t, b_ps, gt, bt, _sl=sl):
                        d = p_tmp.tile([P, 512], f32, tag="d2", bufs=2, name="d2")
                        nc.vector.tensor_tensor(d, t, b_ps, ALU.subtract)
                        nc.vector.tensor_scalar(
                            out=ln1[c][:, _sl], in0=d,
                            scalar1=gt[:, c : c + 1], scalar2=bt[:, c : c + 1],
                            op0=ALU.mult, op1=ALU.add,
                        )

                layer_norm_T(r1v, sb_g1, sb_be1, ln1_out)

            def mlp_pass1(tb):
                sl = slice(tb * 512, (tb + 1) * 512)
                h_sb = []
                for m in range(NB_F):
                    h_ps = p_psm.tile([P, 512], f32, tag="h_ps", bufs=2,
                                      name="h_ps")
                    for k in range(NB_C):
                        nc.tensor.matmul(
                            h_ps,
                            sb_w1[k][:, m * P : (m + 1) * P],
                            ln1[k][:, sl],
                            start=(k == 0),
                            stop=(k == NB_C - 1),
                        )
                    hs = p_h.tile([P, 512], bf16, tag=f"h{m}", name=f"h{m}")
                    nc.scalar.activation(
                        out=hs, in_=h_ps, func=AF.Gelu,
                        bias=sb_b1[:, m : m + 1], scale=1.0,
                    )
                    h_sb.append(hs)
                return h_sb

            def mlp_pass2(tb, h_sb):
                # r2 = W2.T h + b2 + ln1  (single fused DVE op per chunk;
                # result overwrites r1, bf16)
                sl = slice(tb * 512, (tb + 1) * 512)
                r2v = []
                for c in range(NB_C):
                    y_ps = p_psm.tile([P, 512], f32, tag="y_ps", bufs=2,
                                      name="y_ps")
                    for m in range(NB_F):
                        nc.tensor.matmul(
                            y_ps,
                            sb_w2[m][:, c * P : (c + 1) * P],
                            h_sb[m],
                            start=(m == 0),
                            stop=(m == NB_F - 1),
                        )
                    nc.vector.scalar_tensor_tensor(
                        r1[c][:, sl], y_ps, sb_b2[:, c : c + 1],
                        ln1[c][:, sl], ALU.add, ALU.add,
                    )
                    r2v.append(r1[c][:, sl])
                return r2v

            def ln2_emit(tb, r2v):
                sl = slice(tb * 512, (tb + 1) * 512)

                if trivial_gb:
                    def ln2_out(c, t, b_ps, gt, bt, _sl=sl):
                        o = p_out.tile([P, 512], f32, tag="o", bufs=2, name="o")
                        nc.vector.tensor_tensor(o, t, b_ps, ALU.subtract)
                        nc.gpsimd.dma_start(out=outT_t[c][:, _sl], in_=o)
                else:
                    def ln2_out(c, t, b_ps, gt, bt, _sl=sl):
                        d = p_tmp.tile([P, 512], f32, tag="d2", bufs=2, name="d2")
                        nc.vector.tensor_tensor(d, t, b_ps, ALU.subtract)
                        o = p_out.tile([P, 512], f32, tag="o", bufs=2, name="o")
                        nc.vector.tensor_scalar(
                            out=o, in0=d,
                            scalar1=gt[:, c : c + 1], scalar2=bt[:, c : c + 1],
                            op0=ALU.mult, op1=ALU.add,
                        )
                        nc.gpsimd.dma_start(out=outT_t[c][:, _sl], in_=o)

                layer_norm_T(r2v, sb_g2, sb_be2, ln2_out)

            # order chosen to keep the PE stream dense and minimize ACT
            # table swaps: M1(0) M2(0) M1(1) LN2(0) M2(1) LN2(1)
            h0 = mlp_pass1(0)
            r2v0 = mlp_pass2(0, h0)
            h1 = mlp_pass1(1)
            ln2_emit(0, r2v0)
            r2v1 = mlp_pass2(1, h1)
            ln2_emit(1, r2v1)

    return nc


def _spill_excess_waits(nc, maxw=2):
    """walrus (this build) caps sync-wait commands per instruction. Move
    excess waits onto freshly inserted same-engine nops placed immediately
    before the over-limit instruction (same engine stream => the waits
    still complete before it executes)."""
    import copy

    import concourse.bass as bass
    import concourse.mybir as mybir

    scratch = bass.Bass()
    tpl = scratch.sync.nop(nofuse=True).ins
    ctr = [0]

    def mknop(engine, waits):
        n = copy.deepcopy(tpl)
        ctr[0] += 1
        n.name = f"I-spill{ctr[0]}"
        n.engine = engine
        n.sync_info = mybir.SyncInfo(on_wait=list(waits), on_update=[])
        return n

    fn = nc.m.functions[0]
    for bb in fn.blocks:
        changed = False
        out = []
        for inst in bb.instructions:
            si = inst.sync_info
            waits = list(si.on_wait) if si and si.on_wait else []
            nupd = len(si.on_update) if si and si.on_update else 0
            lim = max(0, maxw - nupd)   # waits + updates <= maxw total
            if len(waits) > lim:
                keep = waits[-lim:] if lim else []
                rest = waits[: len(waits) - lim]
                while rest:
                    chunk, rest = rest[:1], rest[1:]
                    out.append(mknop(inst.engine, chunk))
                si.on_wait = keep
                changed = True
            out.append(inst)
        if changed:
            bb.instructions = out
    return nc


def _get_nc(trivial_gb):
    key = ("nc", trivial_gb)
    if key not in _compiled:
        _patch_tile_drain()
        _patch_profile_hook()
        nc = _build_nc(trivial_gb)
        _spill_excess_waits(nc, maxw=2)
        _compiled[key] = nc
    return _compiled[key]


# --------------------------------------------------------------------------
# host-side sharding
# --------------------------------------------------------------------------

def _make_in_maps(x, Wq, Wk, Wv, ln1_g, ln1_b, W1, b1, W2, b2, ln2_g, ln2_b):
    x = np.asarray(x, np.float32)
    wq_s = np.ascontiguousarray(
        np.asarray(Wq, np.float32).transpose(1, 0, 2).reshape(C, C)
    ).astype(BF16)
    wk_s = np.ascontiguousarray(
        np.asarray(Wk, np.float32).transpose(1, 0, 2).reshape(C, C)
    ).astype(BF16)
    wv_s = np.ascontiguousarray(
        np.asarray(Wv, np.float32).transpose(1, 0, 2).reshape(C, C)
    ).astype(BF16)
    w1b = np.asarray(W1, np.float32).astype(BF16)
    w2b = np.asarray(W2, np.float32).astype(BF16)
    b1r = np.ascontiguousarray(np.asarray(b1, np.float32).reshape(NB_F, P).T)
    b2r = np.ascontiguousarray(np.asarray(b2, np.float32).reshape(NB_C, P).T)
    g1r = np.ascontiguousarray(np.asarray(ln1_g, np.float32).reshape(NB_C, P).T)
    be1r = np.ascontiguousarray(np.asarray(ln1_b, np.float32).reshape(NB_C, P).T)
    g2r = np.ascontiguousarray(np.asarray(ln2_g, np.float32).reshape(NB_C, P).T)
    be2r = np.ascontiguousarray(np.asarray(ln2_b, np.float32).reshape(NB_C, P).T)

    in_maps = []
    for core in range(N_CORES):
        b, g = core // 2, core % 2
        xb = x[b]                                # [T, C]
        xTa = np.ascontiguousarray(xb.T)         # [C, T]
        own = np.arange(g, T, 2)
        xo = np.ascontiguousarray(xb[own].T)     # [C, OT]
        ii = np.arange(P)[:, None]
        mm = np.arange(64)[None, :]
        cm = np.where(ii <= 2 * mm + g, 1.0, 0.0).astype(BF16)
        in_maps.append(
            {
                "xT": xTa.astype(BF16),
                "xTo16": xo.astype(BF16),
                "wq": wq_s,
                "wk": wk_s,
                "wv": wv_s,
                "w1": w1b,
                "w2": w2b,
                "b1r": b1r,
                "b2r": b2r,
                "g1r": g1r,
                "be1r": be1r,
                "g2r": g2r,
                "be2r": be2r,
                "cmask": cm,
            }
        )
    return in_maps


def _assemble(results):
    out = np.empty((B, T, C), np.float32)
    for core in range(N_CORES):
        b, g = core // 2, core % 2
        own = np.arange(g, T, 2)
        out[b, own, :] = results[core]["outT"].T
    return out


def kernel(_trace=False, **inputs):
    from concourse.bass_utils import run_bass_kernel_spmd

    trivial_gb = bool(
        np.all(np.asarray(inputs["ln1_g"]) == 1.0)
        and np.all(np.asarray(inputs["ln1_b"]) == 0.0)
        and np.all(np.asarray(inputs["ln2_g"]) == 1.0)
        and np.all(np.asarray(inputs["ln2_b"]) == 0.0)
    )
    nc = _get_nc(trivial_gb)
    in_maps = _make_in_maps(**inputs)
    res = run_bass_kernel_spmd(nc, in_maps, list(range(N_CORES)), trace=_trace)
    out = _assemble(res.results)
    if _trace:
        return out, res
    return out
